# revision 13
# baseline (speedup 1.0000x reference)
"""Trainium2 Bass kernel for nn_FDLT (forward discrete Legendre transform).

Math: for each of the 127 m-blocks, the reference does
    out[:, mi, :] = (Cm[mi] * psiHat[:, mi, :]) @ XF_mi @ Dblk_mi.T
where XF_mi alternates XFc/XFs by mi parity and Dblk_mi is the mi-th
block of the block-diagonal sparse Wigner matrix D.  All tables are
runtime constants, so fold them on the host into A_mi = Cm[mi] * XF_mi
@ Dblk_mi.T (shape [128, 64]) and the device work collapses to 127
independent [512,128]@[128,64] matmuls.

Sharding: m-parallel across 8 cores (16 blocks/core, padded 128 with a
zero block), full batch per core.  The host feeds each core its input
slab pre-transposed to [n, j, b] so the contraction dim n lands on the
SBUF partition axis.  Block pairs write one shared [128, 512] PSUM bank
through PE quadrant placement: even block j -> out rows 0:64 with the
compact [128, 64] stationary at tile column 0, odd j -> rows 64:128 at
tile column 64 (tile_position).  Weights stay compact ([128, 16*64],
256 KB) instead of zero-padded pairs, halving weight DMA and keeping
the weight transfer off the input-stream critical path.  PSUM->SBUF
copies and output stores still run at the full 128 partitions.

Device I/O is fp16 (fp32 PSUM accumulation), measured ~3e-4 relative
error against the fp32 reference.
"""

from contextlib import ExitStack

import numpy as np

import concourse.bacc as bacc
import concourse.bass as bass  # noqa: F401
import concourse.mybir as mybir
from concourse.bass_utils import run_bass_kernel_spmd

P = 128      # SBUF partitions = n dim (2B)
B = 64       # l dim per block
M = 127      # number of m blocks
NB = 512     # full batch
NCORES = 8
JPC = 16     # m-blocks per core (8*16 = 128 = 127 real + 1 zero pad)
PAIRS = JPC // 2
# Input slab schedule (blocks per DMA).  Small head slabs get the first
# completion receipts to the PE sooner (receipts lag the data by
# ~2.5 us); small tail slabs shorten the post-last-byte chain.
SLABS = (1, 1, 2, 2, 2, 2, 2, 2, 1, 1)
WARMUP = 4   # garbage matmuls to ungate the PE clock before real work

# fp16 keeps a 10-bit mantissa (measured ~3e-4 relative error vs the
# fp32 reference with fp32-PSUM accumulation) while halving the DMA
# traffic that bounds this kernel.
DT_IN = mybir.dt.float16

_programs = {}


def _build_raw(dt_in):
    """Raw-bass pipeline with explicit semaphores, emitted blockless.

    Engine roles: Scalar loads the weights + input slab 0, then streams
    the output stores; Sync and GpSimd split the remaining input slabs
    between their two DMA rings; Tensor warms the PE clock gate with
    garbage matmuls, then runs the 16 real matmuls; Vector packs PSUM
    banks into SBUF staging.

    Semaphore soundness: a dma `then_inc(sem, 16)` arrives as 16
    independent +1s (one per SDMA engine), so a single cumulative sem
    across several DMAs can reach 16*k with one slow engine still
    mid-transfer on an early DMA.  Every DMA whose completion anything
    waits on therefore gets its own semaphore (per-slab s_in[i], s_av),
    and PSUM/output staging buffers are not reused (8 pairs = 8 PSUM
    banks + 8 staging tiles), killing all reuse waits.  s_mm/s_cp are
    single-producer compute sems (in-order increments), safe to wait on
    cumulatively.  The kernel ends with a full store-completion wait
    so no DMA is in flight when the NEFF epilogue runs.

    After compile, the unused const-AP memsets of the Bass preamble are
    stripped from the BIR, and each engine's leading run of wait-free
    input DMA issues is hoisted above its init-barrier entry so the
    transfers start while the Tensor sequencer is still waking up.  The
    barrier itself is kept (builds without it intermittently crash the
    device at a later process load).
    """
    assert dt_in != mybir.dt.float32r, "f32r path removed (compact lhsT only)"
    # 16-bit input -> store the output in fp16 too (host upcasts); the
    # extra ~2.4e-4 relative rounding halves the dominant output traffic.
    dt_out = (
        mybir.dt.float16
        if dt_in in (mybir.dt.float16, mybir.dt.bfloat16)
        else mybir.dt.float32
    )

    nc = bacc.Bacc(
        "TRN2", target_bir_lowering=False, debug=False, num_devices=NCORES
    )
    xt = nc.dram_tensor("xt", [P, JPC * NB], dt_in, kind="ExternalInput")
    av = nc.dram_tensor("av", [P, JPC * B], dt_in, kind="ExternalInput")
    out = nc.dram_tensor("out", [P, PAIRS * NB], dt_out, kind="ExternalOutput")

    assert sum(SLABS) == JPC
    NSLAB = len(SLABS)
    slab_of = []  # block j -> slab index
    slab_starts = []
    pos = 0
    for si, w in enumerate(SLABS):
        slab_starts.append(pos)
        slab_of.extend([si] * w)
        pos += w

    with ExitStack() as ctx:
        x_sb = ctx.enter_context(nc.sbuf_tensor("x_sb", [P, JPC * NB], dt_in))
        a_sb = ctx.enter_context(nc.sbuf_tensor("a_sb", [P, JPC * B], dt_in))
        o_sb = [
            ctx.enter_context(nc.sbuf_tensor(f"o_sb{i}", [P, NB], dt_out))
            for i in range(PAIRS)
        ]
        ps = [
            ctx.enter_context(
                nc.psum_tensor(f"ps{i}", [P, NB], mybir.dt.float32)
            )
            for i in range(PAIRS)
        ]
        s_in = [
            ctx.enter_context(nc.semaphore(f"s_in{i}")) for i in range(NSLAB)
        ]
        s_av = ctx.enter_context(nc.semaphore("s_av"))
        s_mm = ctx.enter_context(nc.semaphore("s_mm"))
        s_cpv = ctx.enter_context(nc.semaphore("s_cpv"))
        s_cps = ctx.enter_context(nc.semaphore("s_cps"))
        s_st = ctx.enter_context(nc.semaphore("s_st"))

        # --- Input issue plan: spread the ~0.65 us/DMA issue cost over
        # three DMA-capable engines (three rings) so every slab is issued
        # early and the SDMA engines can stream back-to-back.
        def _slab_dma(eng, s):
            lo = slab_starts[s] * NB
            hi = lo + SLABS[s] * NB
            eng.dma_start(out=x_sb[:, lo:hi], in_=xt[:, lo:hi]).then_inc(
                s_in[s], 16
            )

        nc.scalar.dma_start(out=a_sb[:], in_=av[:]).then_inc(s_av, 16)
        _slab_dma(nc.scalar, 0)
        for s in range(1, NSLAB, 2):
            _slab_dma(nc.sync, s)
        for s in range(2, NSLAB, 2):
            _slab_dma(nc.gpsimd, s)

        # --- Tensor: PE clock warm-up, then the 16 real matmuls.  Each
        # block is a [128, 64] compact stationary placed at PE tile
        # column 0 (even j -> PSUM rows 0:64) or 64 (odd j -> rows
        # 64:128); the pair shares one [128, 512] PSUM bank.
        for w in range(WARMUP):
            nc.tensor.matmul(
                ps[0][0:B, :],
                lhsT=a_sb[:, 0:B],
                rhs=x_sb[:, (JPC - 1) * NB : JPC * NB],
                start=True,
                stop=True,
                tile_position=(0, 0),
                skip_group_check=True,
            )
        nc.tensor.wait_ge(s_av, 16)
        for j in range(JPC):
            if j in slab_starts:
                nc.tensor.wait_ge(s_in[slab_of[j]], 16)
            p = j // 2
            half = ps[p][0:B, :] if j % 2 == 0 else ps[p][B:P, :]
            nc.tensor.matmul(
                half,
                lhsT=a_sb[:, j * B : (j + 1) * B],
                rhs=x_sb[:, j * NB : (j + 1) * NB],
                start=True,
                stop=True,
                tile_position=(0, 0 if j % 2 == 0 else B),
            ).then_inc(s_mm, 1)

        # --- PSUM -> SBUF staging, split across Vector (even pairs, sem
        # s_cpv) and Scalar's activation-copy (odd pairs, sem s_cps) so
        # the cast tail after the last receipt is not serialized on one
        # engine.  (GpSimd cannot read PSUM.)  The last pair is further
        # split into halves so its store can begin while the second half
        # converts.
        lp = PAIRS - 1
        h = NB // 2
        for p in range(PAIRS - 1):
            if p % 2 == 0:
                nc.vector.wait_ge(s_mm, 2 * p + 2)
                nc.vector.tensor_copy(o_sb[p][:], ps[p][:]).then_inc(
                    s_cpv, 1
                )
            else:
                nc.scalar.wait_ge(s_mm, 2 * p + 2)
                nc.scalar.copy(o_sb[p][:], ps[p][:]).then_inc(s_cps, 1)
        nc.scalar.wait_ge(s_mm, 2 * lp + 2)
        nc.scalar.copy(o_sb[lp][:, 0:h], ps[lp][:, 0:h]).then_inc(s_cps, 1)
        nc.scalar.copy(o_sb[lp][:, h:NB], ps[lp][:, h:NB]).then_inc(s_cps, 1)

        # --- Output stores, split across Sync (even pairs) and GpSimd
        # (odd pairs); the last pair's two half-stores are issued from
        # both engines in parallel.
        for p in range(PAIRS - 1):
            if p % 2 == 0:
                nc.sync.wait_ge(s_cpv, p // 2 + 1)
                eng = nc.sync
            else:
                nc.gpsimd.wait_ge(s_cps, (p + 1) // 2)
                eng = nc.gpsimd
            eng.dma_start(
                out=out[:, p * NB : (p + 1) * NB], in_=o_sb[p][:]
            ).then_inc(s_st, 16)
        sc_n = (PAIRS - 1) // 2  # scalar full-pair casts before the halves
        nc.gpsimd.wait_ge(s_cps, sc_n + 1)
        nc.gpsimd.dma_start(
            out=out[:, lp * NB : lp * NB + h], in_=o_sb[lp][:, 0:h]
        ).then_inc(s_st, 16)
        nc.sync.wait_ge(s_cps, sc_n + 2)
        nc.sync.dma_start(
            out=out[:, lp * NB + h : (lp + 1) * NB], in_=o_sb[lp][:, h:NB]
        ).then_inc(s_st, 16)
        # Wait for every store to land before the kernel ends so no DMA
        # is in flight when the NEFF epilogue runs.
        nc.scalar.wait_ge(s_st, 16 * (PAIRS + 1))

    nc.compile()

    # Strip only the unused const-AP memsets from the Bass preamble.  The
    # init all-engine barrier MUST stay: builds without it intermittently
    # leave the device unrecoverable at a subsequent fresh-process load
    # (~1-in-6 launches, observed twice), even with the store-quiesce
    # wait in place.
    for blk in nc.m.functions[0].blocks:
        blk.instructions = [
            i for i in blk.instructions if getattr(i, "opcode", "") != "Memset"
        ]

    # Hoist each engine's leading run of wait-free issues (DMA doorbells,
    # PE warm-up matmuls) above its init-barrier SEMAPHORE but below its
    # barrier DRAIN.  Drain blocks until the engine's outstanding DMAs
    # retire, so the issues must come after it; the barrier semaphore is
    # a pure sequencer sync, so work issued before it overlaps the slow
    # Tensor-sequencer wake-up that the barrier otherwise serializes
    # behind.  The semaphores the DMAs increment are initialized by the
    # NEFF loader, not by in-program clears, so pre-barrier issue is
    # sound.
    for blk in nc.m.functions[0].blocks:
        insts = blk.instructions
        per_eng = {}  # engine -> ordered instruction indices
        for idx, ins in enumerate(insts):
            eng = getattr(ins, "engine", None)
            if eng is not None:
                per_eng.setdefault(eng, []).append(idx)
        moved = []  # (insert_before_idx, [hoisted indices])
        for eng, idxs in per_eng.items():
            k = 0
            first_bar_sem = None
            # Skip the init-barrier prefix: Drain + barrier EventSemaphores.
            while k < len(idxs):
                ins = insts[idxs[k]]
                op = getattr(ins, "opcode", "")
                nm = getattr(ins, "name", "") or ""
                if op == "Drain":
                    k += 1
                elif op == "EventSemaphore" and nm.startswith("barrier_"):
                    if first_bar_sem is None:
                        first_bar_sem = idxs[k]
                    k += 1
                else:
                    break
            if first_bar_sem is None:
                continue  # no barrier prefix for this engine
            run = []
            while k < len(idxs):
                ins = insts[idxs[k]]
                si = getattr(ins, "sync_info", None)
                has_wait = si is not None and len(si.on_wait) > 0
                if (
                    getattr(ins, "opcode", "") in ("DMACopy", "Matmult")
                    and not has_wait
                ):
                    run.append(idxs[k])
                    k += 1
                else:
                    break
            if run:
                moved.append((first_bar_sem, run))
        if moved:
            pulled = {i for _, run in moved for i in run}
            inserts = {bidx: run for bidx, run in moved}
            new = []
            for idx, ins in enumerate(insts):
                if idx in pulled:
                    continue
                if idx in inserts:
                    new.extend(insts[i] for i in inserts[idx])
                new.append(ins)
            blk.instructions = new
    return nc


def _get_program(dt_in, raw=True):
    key = (str(dt_in),)
    if key not in _programs:
        _programs[key] = _build_raw(dt_in)
    return _programs[key]


def _fold_tables(Cm, XFc, XFs, D_val, D_row, D_col):
    """A[mi] = Cm[mi] * XF_mi @ Dblk_mi.T in float64 -> [128, 128, 64]."""
    Cm = np.asarray(Cm, np.float64)
    XFc = np.asarray(XFc, np.float64)
    XFs = np.asarray(XFs, np.float64)
    vals = np.asarray(D_val, np.float64)
    rows = np.asarray(D_row, np.int64)
    cols = np.asarray(D_col, np.int64)

    mi = rows // B
    l = rows - mi * B
    n = cols - mi * (2 * B)
    Dt = np.zeros((M, 2 * B, B))  # [mi, n, l] = Dblk_mi.T
    Dt[mi, n, l] = vals

    A = np.zeros((P, P, B))  # padded to 128 blocks; A[127] stays 0
    # B-1 = 63 is odd -> cos rows are the odd mi, sin rows the even mi
    A[0:M:2] = np.einsum("nk,mkl->mnl", XFs, Dt[0::2], optimize=True)
    A[1:M:2] = np.einsum("nk,mkl->mnl", XFc, Dt[1::2], optimize=True)
    A[:M] *= Cm[:, None, None]
    return A


def _np_dtype(dt_in):
    return mybir.dt.np(dt_in)


def _run(psiHat, A, trace=False, dt_in=DT_IN, raw=True):
    dt_np = _np_dtype(dt_in)
    # [b, m, n] -> [m, n, b], contiguous
    PT = np.ascontiguousarray(psiHat.transpose(1, 2, 0).astype(np.float32))

    in_maps = []
    for k in range(NCORES):
        mi0 = JPC * k
        nj = min(JPC, M - mi0)
        xt_k = np.zeros((P, JPC, NB), dt_np)
        xt_k[:, :nj, :] = PT[mi0 : mi0 + nj].transpose(1, 0, 2)
        a_k = np.zeros((P, JPC, B), dt_np)
        a_k[:, :nj, :] = A[mi0 : mi0 + nj].transpose(1, 0, 2)  # [n, nj, 64]
        in_maps.append(
            {"xt": xt_k.reshape(P, JPC * NB), "av": a_k.reshape(P, JPC * B)}
        )

    nc = _get_program(dt_in)
    res = run_bass_kernel_spmd(nc, in_maps, list(range(NCORES)), trace=trace)

    out = np.empty((NB, M, B), np.float32)
    for k in range(NCORES):
        mi0 = JPC * k
        nj = min(JPC, M - mi0)
        o = np.asarray(res.results[k]["out"]).reshape(2, B, PAIRS, NB)  # [h,l,p,b]
        ot = o.transpose(2, 0, 1, 3).reshape(JPC, B, NB)  # [j, l, b]
        out[:, mi0 : mi0 + nj, :] = ot[:nj].transpose(2, 0, 1)
    return out, res.exec_time_ns


def kernel(psiHat, Cm, XFc, XFs, D_val, D_row, D_col):
    psiHat = np.asarray(psiHat)
    A = _fold_tables(Cm, XFc, XFs, D_val, D_row, D_col)
    return _run(psiHat, A, trace=False)[0]


# revision 14
# speedup vs baseline: 1.1381x; 1.1381x over previous
"""Trainium2 Bass kernel for nn_FDLT (forward discrete Legendre transform).

Math: for each of the 127 m-blocks, the reference does
    out[:, mi, :] = (Cm[mi] * psiHat[:, mi, :]) @ XF_mi @ Dblk_mi.T
where XF_mi alternates XFc/XFs by mi parity and Dblk_mi is the mi-th
block of the block-diagonal sparse Wigner matrix D.  All tables are
runtime constants, so fold them on the host into A_mi = Cm[mi] * XF_mi
@ Dblk_mi.T (shape [128, 64]) and the device work collapses to 127
independent [512,128]@[128,64] matmuls.

Sharding: m-parallel across 8 cores (16 blocks/core, padded 128 with a
zero block), full batch per core.  The host feeds each core its input
slab pre-transposed to [n, j, b] so the contraction dim n lands on the
SBUF partition axis.  Block pairs write one shared [128, 512] PSUM bank
through PE quadrant placement: even block j -> out rows 0:64 with the
compact [128, 64] stationary at tile column 0, odd j -> rows 64:128 at
tile column 64 (tile_position).  Weights stay compact ([128, 16*64],
256 KB) instead of zero-padded pairs, halving weight DMA and keeping
the weight transfer off the input-stream critical path.  PSUM->SBUF
copies and output stores still run at the full 128 partitions.

Device I/O is fp16 (fp32 PSUM accumulation), measured ~3e-4 relative
error against the fp32 reference.
"""

from contextlib import ExitStack

import numpy as np

import concourse.bacc as bacc
import concourse.bass as bass  # noqa: F401
import concourse.mybir as mybir
from concourse.bass_utils import run_bass_kernel_spmd

P = 128      # SBUF partitions = n dim (2B)
B = 64       # l dim per block
M = 127      # number of m blocks
NB = 512     # full batch
NCORES = 8
JPC = 16     # m-blocks per core (8*16 = 128 = 127 real + 1 zero pad)
PAIRS = JPC // 2
# Input slab schedule (blocks per DMA).  Small head slabs get the first
# completion receipts to the PE sooner (receipts lag the data by
# ~2.5 us); small tail slabs shorten the post-last-byte chain.
SLABS = (1, 1, 2, 2, 2, 2, 2, 2, 1, 1)
WARMUP = 4   # garbage matmuls to ungate the PE clock before real work

# fp16 keeps a 10-bit mantissa (measured ~3e-4 relative error vs the
# fp32 reference with fp32-PSUM accumulation) while halving the DMA
# traffic that bounds this kernel.
DT_IN = mybir.dt.float16

_programs = {}


def _build_raw(dt_in):
    """Raw-bass pipeline with explicit semaphores, emitted blockless.

    Engine roles: Scalar loads the weights + input slab 0, then streams
    the output stores; Sync and GpSimd split the remaining input slabs
    between their two DMA rings; Tensor warms the PE clock gate with
    garbage matmuls, then runs the 16 real matmuls; Vector packs PSUM
    banks into SBUF staging.

    Semaphore soundness: a dma `then_inc(sem, 16)` arrives as 16
    independent +1s (one per SDMA engine), so a single cumulative sem
    across several DMAs can reach 16*k with one slow engine still
    mid-transfer on an early DMA.  Every DMA whose completion anything
    waits on therefore gets its own semaphore (per-slab s_in[i], s_av),
    and PSUM/output staging buffers are not reused (8 pairs = 8 PSUM
    banks + 8 staging tiles), killing all reuse waits.  s_mm/s_cp are
    single-producer compute sems (in-order increments), safe to wait on
    cumulatively.  The kernel ends with a full store-completion wait
    so no DMA is in flight when the NEFF epilogue runs.

    After compile, the unused const-AP memsets of the Bass preamble are
    stripped from the BIR, and each engine's leading run of wait-free
    input DMA issues is hoisted above its init-barrier entry so the
    transfers start while the Tensor sequencer is still waking up.  The
    barrier itself is kept (builds without it intermittently crash the
    device at a later process load).
    """
    assert dt_in != mybir.dt.float32r, "f32r path removed (compact lhsT only)"
    # 16-bit input -> store the output in fp16 too (host upcasts); the
    # extra ~2.4e-4 relative rounding halves the dominant output traffic.
    dt_out = (
        mybir.dt.float16
        if dt_in in (mybir.dt.float16, mybir.dt.bfloat16)
        else mybir.dt.float32
    )

    nc = bacc.Bacc(
        "TRN2", target_bir_lowering=False, debug=False, num_devices=NCORES
    )
    xt = nc.dram_tensor("xt", [P, JPC * NB], dt_in, kind="ExternalInput")
    av = nc.dram_tensor("av", [P, JPC * B], dt_in, kind="ExternalInput")
    out = nc.dram_tensor("out", [P, PAIRS * NB], dt_out, kind="ExternalOutput")

    assert sum(SLABS) == JPC
    NSLAB = len(SLABS)
    slab_of = []  # block j -> slab index
    slab_starts = []
    pos = 0
    for si, w in enumerate(SLABS):
        slab_starts.append(pos)
        slab_of.extend([si] * w)
        pos += w

    with ExitStack() as ctx:
        x_sb = ctx.enter_context(nc.sbuf_tensor("x_sb", [P, JPC * NB], dt_in))
        a_sb = ctx.enter_context(nc.sbuf_tensor("a_sb", [P, JPC * B], dt_in))
        o_sb = [
            ctx.enter_context(nc.sbuf_tensor(f"o_sb{i}", [P, NB], dt_out))
            for i in range(PAIRS)
        ]
        ps = [
            ctx.enter_context(
                nc.psum_tensor(f"ps{i}", [P, NB], mybir.dt.float32)
            )
            for i in range(PAIRS)
        ]
        s_in = [
            ctx.enter_context(nc.semaphore(f"s_in{i}")) for i in range(NSLAB)
        ]
        s_av = ctx.enter_context(nc.semaphore("s_av"))
        s_mm = ctx.enter_context(nc.semaphore("s_mm"))
        s_cpv = ctx.enter_context(nc.semaphore("s_cpv"))
        s_cps = ctx.enter_context(nc.semaphore("s_cps"))
        s_st = ctx.enter_context(nc.semaphore("s_st"))

        # --- Input issue plan: spread the ~0.65 us/DMA issue cost over
        # three DMA-capable engines (three rings) so every slab is issued
        # early and the SDMA engines can stream back-to-back.
        def _slab_dma(eng, s):
            lo = slab_starts[s] * NB
            hi = lo + SLABS[s] * NB
            eng.dma_start(out=x_sb[:, lo:hi], in_=xt[:, lo:hi]).then_inc(
                s_in[s], 16
            )

        nc.scalar.dma_start(out=a_sb[:], in_=av[:]).then_inc(s_av, 16)
        _slab_dma(nc.scalar, 0)
        for s in range(1, NSLAB, 2):
            _slab_dma(nc.sync, s)
        for s in range(2, NSLAB, 2):
            _slab_dma(nc.gpsimd, s)

        # --- Tensor: PE clock warm-up, then the 16 real matmuls.  Each
        # block is a [128, 64] compact stationary placed at PE tile
        # column 0 (even j -> PSUM rows 0:64) or 64 (odd j -> rows
        # 64:128); the pair shares one [128, 512] PSUM bank.
        for w in range(WARMUP):
            nc.tensor.matmul(
                ps[0][0:B, :],
                lhsT=a_sb[:, 0:B],
                rhs=x_sb[:, (JPC - 1) * NB : JPC * NB],
                start=True,
                stop=True,
                tile_position=(0, 0),
                skip_group_check=True,
            )
        nc.tensor.wait_ge(s_av, 16)
        for j in range(JPC):
            if j in slab_starts:
                nc.tensor.wait_ge(s_in[slab_of[j]], 16)
            p = j // 2
            half = ps[p][0:B, :] if j % 2 == 0 else ps[p][B:P, :]
            nc.tensor.matmul(
                half,
                lhsT=a_sb[:, j * B : (j + 1) * B],
                rhs=x_sb[:, j * NB : (j + 1) * NB],
                start=True,
                stop=True,
                tile_position=(0, 0 if j % 2 == 0 else B),
            ).then_inc(s_mm, 1)

        # --- PSUM -> SBUF staging, split across Vector (even pairs, sem
        # s_cpv) and Scalar's activation-copy (odd pairs, sem s_cps) so
        # the cast tail after the last receipt is not serialized on one
        # engine.  (GpSimd cannot read PSUM.)  The last pair is further
        # split into halves so its store can begin while the second half
        # converts.
        lp = PAIRS - 1
        h = NB // 2
        for p in range(PAIRS - 1):
            if p % 2 == 0:
                nc.vector.wait_ge(s_mm, 2 * p + 2)
                nc.vector.tensor_copy(o_sb[p][:], ps[p][:]).then_inc(
                    s_cpv, 1
                )
            else:
                nc.scalar.wait_ge(s_mm, 2 * p + 2)
                nc.scalar.copy(o_sb[p][:], ps[p][:]).then_inc(s_cps, 1)
        nc.scalar.wait_ge(s_mm, 2 * lp + 2)
        nc.scalar.copy(o_sb[lp][:, 0:h], ps[lp][:, 0:h]).then_inc(s_cps, 1)
        nc.scalar.copy(o_sb[lp][:, h:NB], ps[lp][:, h:NB]).then_inc(s_cps, 1)

        # --- Output stores, split across Sync (even pairs) and GpSimd
        # (odd pairs); the last pair's two half-stores are issued from
        # both engines in parallel.
        for p in range(PAIRS - 1):
            if p % 2 == 0:
                nc.sync.wait_ge(s_cpv, p // 2 + 1)
                eng = nc.sync
            else:
                nc.gpsimd.wait_ge(s_cps, (p + 1) // 2)
                eng = nc.gpsimd
            eng.dma_start(
                out=out[:, p * NB : (p + 1) * NB], in_=o_sb[p][:]
            ).then_inc(s_st, 16)
        sc_n = (PAIRS - 1) // 2  # scalar full-pair casts before the halves
        nc.gpsimd.wait_ge(s_cps, sc_n + 1)
        nc.gpsimd.dma_start(
            out=out[:, lp * NB : lp * NB + h], in_=o_sb[lp][:, 0:h]
        ).then_inc(s_st, 16)
        nc.sync.wait_ge(s_cps, sc_n + 2)
        nc.sync.dma_start(
            out=out[:, lp * NB + h : (lp + 1) * NB], in_=o_sb[lp][:, h:NB]
        ).then_inc(s_st, 16)
        # Wait for every store to land before the kernel ends so no DMA
        # is in flight when the NEFF epilogue runs.
        nc.scalar.wait_ge(s_st, 16 * (PAIRS + 1))

    nc.compile()

    # Strip only the unused const-AP memsets from the Bass preamble.  The
    # init all-engine barrier MUST stay: builds without it intermittently
    # leave the device unrecoverable at a subsequent fresh-process load
    # (~1-in-6 launches, observed twice), even with the store-quiesce
    # wait in place.
    for blk in nc.m.functions[0].blocks:
        blk.instructions = [
            i for i in blk.instructions if getattr(i, "opcode", "") != "Memset"
        ]

    # Hoist each engine's leading run of wait-free issues (DMA doorbells,
    # PE warm-up matmuls) above its init-barrier SEMAPHORE but below its
    # barrier DRAIN.  Drain blocks until the engine's outstanding DMAs
    # retire, so the issues must come after it; the barrier semaphore is
    # a pure sequencer sync, so work issued before it overlaps the slow
    # Tensor-sequencer wake-up that the barrier otherwise serializes
    # behind.  The semaphores the DMAs increment are initialized by the
    # NEFF loader, not by in-program clears, so pre-barrier issue is
    # sound.
    for blk in nc.m.functions[0].blocks:
        insts = blk.instructions
        per_eng = {}  # engine -> ordered instruction indices
        for idx, ins in enumerate(insts):
            eng = getattr(ins, "engine", None)
            if eng is not None:
                per_eng.setdefault(eng, []).append(idx)
        moved = []  # (insert_before_idx, [hoisted indices])
        for eng, idxs in per_eng.items():
            k = 0
            first_bar_sem = None
            # Skip the init-barrier prefix: Drain + barrier EventSemaphores.
            while k < len(idxs):
                ins = insts[idxs[k]]
                op = getattr(ins, "opcode", "")
                nm = getattr(ins, "name", "") or ""
                if op == "Drain":
                    k += 1
                elif op == "EventSemaphore" and nm.startswith("barrier_"):
                    if first_bar_sem is None:
                        first_bar_sem = idxs[k]
                    k += 1
                else:
                    break
            if first_bar_sem is None:
                continue  # no barrier prefix for this engine
            run = []
            while k < len(idxs):
                ins = insts[idxs[k]]
                si = getattr(ins, "sync_info", None)
                has_wait = si is not None and len(si.on_wait) > 0
                if (
                    getattr(ins, "opcode", "")
                    in ("DMACopy", "Matmult", "LoadActFuncSet")
                    and not has_wait
                ):
                    run.append(idxs[k])
                    k += 1
                else:
                    break
            if run:
                moved.append((first_bar_sem, run))
        if moved:
            pulled = {i for _, run in moved for i in run}
            inserts = {bidx: run for bidx, run in moved}
            new = []
            for idx, ins in enumerate(insts):
                if idx in pulled:
                    continue
                if idx in inserts:
                    new.extend(insts[i] for i in inserts[idx])
                new.append(ins)
            blk.instructions = new
    return nc


def _get_program(dt_in, raw=True):
    key = (str(dt_in),)
    if key not in _programs:
        _programs[key] = _build_raw(dt_in)
    return _programs[key]


def _fold_tables(Cm, XFc, XFs, D_val, D_row, D_col):
    """A[mi] = Cm[mi] * XF_mi @ Dblk_mi.T in float64 -> [128, 128, 64]."""
    Cm = np.asarray(Cm, np.float64)
    XFc = np.asarray(XFc, np.float64)
    XFs = np.asarray(XFs, np.float64)
    vals = np.asarray(D_val, np.float64)
    rows = np.asarray(D_row, np.int64)
    cols = np.asarray(D_col, np.int64)

    mi = rows // B
    l = rows - mi * B
    n = cols - mi * (2 * B)
    Dt = np.zeros((M, 2 * B, B))  # [mi, n, l] = Dblk_mi.T
    Dt[mi, n, l] = vals

    A = np.zeros((P, P, B))  # padded to 128 blocks; A[127] stays 0
    # B-1 = 63 is odd -> cos rows are the odd mi, sin rows the even mi
    A[0:M:2] = np.einsum("nk,mkl->mnl", XFs, Dt[0::2], optimize=True)
    A[1:M:2] = np.einsum("nk,mkl->mnl", XFc, Dt[1::2], optimize=True)
    A[:M] *= Cm[:, None, None]
    return A


def _np_dtype(dt_in):
    return mybir.dt.np(dt_in)


def _run(psiHat, A, trace=False, dt_in=DT_IN, raw=True):
    dt_np = _np_dtype(dt_in)
    # [b, m, n] -> [m, n, b], contiguous
    PT = np.ascontiguousarray(psiHat.transpose(1, 2, 0).astype(np.float32))

    in_maps = []
    for k in range(NCORES):
        mi0 = JPC * k
        nj = min(JPC, M - mi0)
        xt_k = np.zeros((P, JPC, NB), dt_np)
        xt_k[:, :nj, :] = PT[mi0 : mi0 + nj].transpose(1, 0, 2)
        a_k = np.zeros((P, JPC, B), dt_np)
        a_k[:, :nj, :] = A[mi0 : mi0 + nj].transpose(1, 0, 2)  # [n, nj, 64]
        in_maps.append(
            {"xt": xt_k.reshape(P, JPC * NB), "av": a_k.reshape(P, JPC * B)}
        )

    nc = _get_program(dt_in)
    res = run_bass_kernel_spmd(nc, in_maps, list(range(NCORES)), trace=trace)

    out = np.empty((NB, M, B), np.float32)
    for k in range(NCORES):
        mi0 = JPC * k
        nj = min(JPC, M - mi0)
        o = np.asarray(res.results[k]["out"]).reshape(2, B, PAIRS, NB)  # [h,l,p,b]
        ot = o.transpose(2, 0, 1, 3).reshape(JPC, B, NB)  # [j, l, b]
        out[:, mi0 : mi0 + nj, :] = ot[:nj].transpose(2, 0, 1)
    return out, res.exec_time_ns


def kernel(psiHat, Cm, XFc, XFs, D_val, D_row, D_col):
    psiHat = np.asarray(psiHat)
    A = _fold_tables(Cm, XFc, XFs, D_val, D_row, D_col)
    return _run(psiHat, A, trace=False)[0]


# revision 21
# speedup vs baseline: 1.1621x; 1.0211x over previous
"""Trainium2 Bass kernel for nn_FDLT (forward discrete Legendre transform).

Math: for each of the 127 m-blocks, the reference does
    out[:, mi, :] = (Cm[mi] * psiHat[:, mi, :]) @ XF_mi @ Dblk_mi.T
where XF_mi alternates XFc/XFs by mi parity and Dblk_mi is the mi-th
block of the block-diagonal sparse Wigner matrix D.  All tables are
runtime constants, so fold them on the host into A_mi = Cm[mi] * XF_mi
@ Dblk_mi.T (shape [128, 64]) and the device work collapses to 127
independent [512,128]@[128,64] matmuls.

Sharding: m-parallel across 8 cores (16 blocks/core, padded 128 with a
zero block), full batch per core.  The host feeds each core its input
slab pre-transposed to [n, j, b] so the contraction dim n lands on the
SBUF partition axis.  Block pairs write one shared [128, 512] PSUM bank
through PE quadrant placement: even block j -> out rows 0:64 with the
compact [128, 64] stationary at tile column 0, odd j -> rows 64:128 at
tile column 64 (tile_position).  Weights stay compact ([128, 16*64],
256 KB) instead of zero-padded pairs, halving weight DMA and keeping
the weight transfer off the input-stream critical path.  PSUM->SBUF
copies and output stores still run at the full 128 partitions.

Device I/O is fp16 (fp32 PSUM accumulation), measured ~3e-4 relative
error against the fp32 reference.
"""

from contextlib import ExitStack

import numpy as np

import concourse.bacc as bacc
import concourse.bass as bass  # noqa: F401
import concourse.mybir as mybir
from concourse.bass_utils import run_bass_kernel_spmd

P = 128      # SBUF partitions = n dim (2B)
B = 64       # l dim per block
M = 127      # number of m blocks
NB = 512     # full batch
NCORES = 8
JPC = 16     # m-blocks per core (8*16 = 128 = 127 real + 1 zero pad)
PAIRS = JPC // 2
# Input slab schedule (blocks per DMA).  Small head slabs get the first
# completion receipts to the PE sooner (receipts lag the data by
# ~2.5 us); small tail slabs shorten the post-last-byte chain.
SLABS = (1, 1, 2, 2, 2, 2, 2, 2, 1, 1)
WARMUP = 4   # garbage matmuls to ungate the PE clock before real work

# fp16 keeps a 10-bit mantissa (measured ~3e-4 relative error vs the
# fp32 reference with fp32-PSUM accumulation) while halving the DMA
# traffic that bounds this kernel.
DT_IN = mybir.dt.float16

_programs = {}


def _build_raw(dt_in):
    """Raw-bass pipeline with explicit semaphores, emitted blockless.

    Engine roles: Scalar loads the weights + input slab 0, then streams
    the output stores; Sync and GpSimd split the remaining input slabs
    between their two DMA rings; Tensor warms the PE clock gate with
    garbage matmuls, then runs the 16 real matmuls; Vector packs PSUM
    banks into SBUF staging.

    Semaphore soundness: a dma `then_inc(sem, 16)` arrives as 16
    independent +1s (one per SDMA engine), so a single cumulative sem
    across several DMAs can reach 16*k with one slow engine still
    mid-transfer on an early DMA.  Every DMA whose completion anything
    waits on therefore gets its own semaphore (per-slab s_in[i], s_av),
    and PSUM/output staging buffers are not reused (8 pairs = 8 PSUM
    banks + 8 staging tiles), killing all reuse waits.  s_mm/s_cp are
    single-producer compute sems (in-order increments), safe to wait on
    cumulatively.  The kernel ends with a full store-completion wait
    so no DMA is in flight when the NEFF epilogue runs.

    After compile, the unused const-AP memsets of the Bass preamble are
    stripped from the BIR, and each engine's leading run of wait-free
    input DMA issues is hoisted above its init-barrier entry so the
    transfers start while the Tensor sequencer is still waking up.  The
    barrier itself is kept (builds without it intermittently crash the
    device at a later process load).
    """
    assert dt_in != mybir.dt.float32r, "f32r path removed (compact lhsT only)"
    # 16-bit input -> store the output in fp16 too (host upcasts); the
    # extra ~2.4e-4 relative rounding halves the dominant output traffic.
    dt_out = (
        mybir.dt.float16
        if dt_in in (mybir.dt.float16, mybir.dt.bfloat16)
        else mybir.dt.float32
    )

    nc = bacc.Bacc(
        "TRN2", target_bir_lowering=False, debug=False, num_devices=NCORES
    )
    xt = nc.dram_tensor("xt", [P, JPC * NB], dt_in, kind="ExternalInput")
    av = nc.dram_tensor("av", [P, JPC * B], dt_in, kind="ExternalInput")
    out = nc.dram_tensor("out", [P, PAIRS * NB], dt_out, kind="ExternalOutput")

    assert sum(SLABS) == JPC
    NSLAB = len(SLABS)
    slab_of = []  # block j -> slab index
    slab_starts = []
    pos = 0
    for si, w in enumerate(SLABS):
        slab_starts.append(pos)
        slab_of.extend([si] * w)
        pos += w

    with ExitStack() as ctx:
        x_sb = ctx.enter_context(nc.sbuf_tensor("x_sb", [P, JPC * NB], dt_in))
        a_sb = ctx.enter_context(nc.sbuf_tensor("a_sb", [P, JPC * B], dt_in))
        o_sb = [
            ctx.enter_context(nc.sbuf_tensor(f"o_sb{i}", [P, NB], dt_out))
            for i in range(PAIRS)
        ]
        ps = [
            ctx.enter_context(
                nc.psum_tensor(f"ps{i}", [P, NB], mybir.dt.float32)
            )
            for i in range(PAIRS)
        ]
        s_in = [
            ctx.enter_context(nc.semaphore(f"s_in{i}")) for i in range(NSLAB)
        ]
        s_av = ctx.enter_context(nc.semaphore("s_av"))
        s_mm = ctx.enter_context(nc.semaphore("s_mm"))
        s_cpv = ctx.enter_context(nc.semaphore("s_cpv"))
        s_cps = ctx.enter_context(nc.semaphore("s_cps"))
        s_st = ctx.enter_context(nc.semaphore("s_st"))

        # --- Input issue plan: spread the ~0.65 us/DMA issue cost over
        # three DMA-capable engines (three rings) so every slab is issued
        # early and the SDMA engines can stream back-to-back.
        def _slab_dma(eng, s):
            lo = slab_starts[s] * NB
            hi = lo + SLABS[s] * NB
            eng.dma_start(out=x_sb[:, lo:hi], in_=xt[:, lo:hi]).then_inc(
                s_in[s], 16
            )

        nc.scalar.dma_start(out=a_sb[:], in_=av[:]).then_inc(s_av, 16)
        _slab_dma(nc.scalar, 0)
        for s in range(1, NSLAB, 2):
            _slab_dma(nc.sync, s)
        for s in range(2, NSLAB, 2):
            _slab_dma(nc.gpsimd, s)

        # --- Tensor: PE clock warm-up, then the 16 real matmuls.  Each
        # block is a [128, 64] compact stationary placed at PE tile
        # column 0 (even j -> PSUM rows 0:64) or 64 (odd j -> rows
        # 64:128); the pair shares one [128, 512] PSUM bank.
        for w in range(WARMUP):
            nc.tensor.matmul(
                ps[0][0:B, :],
                lhsT=a_sb[:, 0:B],
                rhs=x_sb[:, (JPC - 1) * NB : JPC * NB],
                start=True,
                stop=True,
                tile_position=(0, 0),
                skip_group_check=True,
            )
        nc.tensor.wait_ge(s_av, 16)
        for j in range(JPC):
            if j in slab_starts:
                nc.tensor.wait_ge(s_in[slab_of[j]], 16)
            p = j // 2
            half = ps[p][0:B, :] if j % 2 == 0 else ps[p][B:P, :]
            nc.tensor.matmul(
                half,
                lhsT=a_sb[:, j * B : (j + 1) * B],
                rhs=x_sb[:, j * NB : (j + 1) * NB],
                start=True,
                stop=True,
                tile_position=(0, 0 if j % 2 == 0 else B),
            ).then_inc(s_mm, 1)

        # --- PSUM -> SBUF staging, split across Vector (even pairs, sem
        # s_cpv) and Scalar's activation-copy (odd pairs, sem s_cps) so
        # the cast tail after the last receipt is not serialized on one
        # engine.  (GpSimd cannot read PSUM.)  The last pair is further
        # split into halves so its store can begin while the second half
        # converts.
        lp = PAIRS - 1
        h = NB // 2
        for p in range(PAIRS - 1):
            if p % 2 == 0:
                nc.vector.wait_ge(s_mm, 2 * p + 2)
                nc.vector.tensor_copy(o_sb[p][:], ps[p][:]).then_inc(
                    s_cpv, 1
                )
            else:
                nc.scalar.wait_ge(s_mm, 2 * p + 2)
                nc.scalar.copy(o_sb[p][:], ps[p][:]).then_inc(s_cps, 1)
        nc.scalar.wait_ge(s_mm, 2 * lp + 2)
        nc.scalar.copy(o_sb[lp][:, 0:h], ps[lp][:, 0:h]).then_inc(s_cps, 1)
        nc.scalar.copy(o_sb[lp][:, h:NB], ps[lp][:, h:NB]).then_inc(s_cps, 1)

        # --- Output stores, split across Sync (even pairs) and GpSimd
        # (odd pairs); the last pair's two half-stores are issued from
        # both engines in parallel.
        for p in range(PAIRS - 1):
            if p % 2 == 0:
                nc.sync.wait_ge(s_cpv, p // 2 + 1)
                eng = nc.sync
            else:
                nc.gpsimd.wait_ge(s_cps, (p + 1) // 2)
                eng = nc.gpsimd
            eng.dma_start(
                out=out[:, p * NB : (p + 1) * NB], in_=o_sb[p][:]
            ).then_inc(s_st, 16)
        sc_n = (PAIRS - 1) // 2  # scalar full-pair casts before the halves
        nc.gpsimd.wait_ge(s_cps, sc_n + 1)
        nc.gpsimd.dma_start(
            out=out[:, lp * NB : lp * NB + h], in_=o_sb[lp][:, 0:h]
        ).then_inc(s_st, 16)
        nc.sync.wait_ge(s_cps, sc_n + 2)
        nc.sync.dma_start(
            out=out[:, lp * NB + h : (lp + 1) * NB], in_=o_sb[lp][:, h:NB]
        ).then_inc(s_st, 16)
        # Wait for every store to land before the kernel ends so no DMA
        # is in flight when the NEFF epilogue runs.  (Tried relying on
        # the NEFF epilogue's fini Drains instead: the exec dies with a
        # runtime INTERNAL error, so the explicit quiesce is required.)
        nc.scalar.wait_ge(s_st, 16 * (PAIRS + 1))

    nc.compile()

    # Strip only the unused const-AP memsets from the Bass preamble.  The
    # init all-engine barrier MUST stay: builds without it intermittently
    # leave the device unrecoverable at a subsequent fresh-process load
    # (~1-in-6 launches, observed twice), even with the store-quiesce
    # wait in place.
    for blk in nc.m.functions[0].blocks:
        blk.instructions = [
            i for i in blk.instructions if getattr(i, "opcode", "") != "Memset"
        ]

    # Hoist each engine's leading run of wait-free issues (DMA doorbells,
    # PE warm-up matmuls) above its init-barrier SEMAPHORE but below its
    # barrier DRAIN.  Drain blocks until the engine's outstanding DMAs
    # retire, so the issues must come after it; the barrier semaphore is
    # a pure sequencer sync, so work issued before it overlaps the slow
    # Tensor-sequencer wake-up that the barrier otherwise serializes
    # behind.  The semaphores the DMAs increment are initialized by the
    # NEFF loader, not by in-program clears, so pre-barrier issue is
    # sound.
    for blk in nc.m.functions[0].blocks:
        insts = blk.instructions
        per_eng = {}  # engine -> ordered instruction indices
        for idx, ins in enumerate(insts):
            eng = getattr(ins, "engine", None)
            if eng is not None:
                per_eng.setdefault(eng, []).append(idx)
        moved = []  # (insert_before_idx, [hoisted indices])
        for eng, idxs in per_eng.items():
            k = 0
            first_bar_sem = None
            # Skip the init-barrier prefix: Drain + barrier EventSemaphores.
            while k < len(idxs):
                ins = insts[idxs[k]]
                op = getattr(ins, "opcode", "")
                nm = getattr(ins, "name", "") or ""
                if op == "Drain":
                    k += 1
                elif op == "EventSemaphore" and nm.startswith("barrier_"):
                    if first_bar_sem is None:
                        first_bar_sem = idxs[k]
                    k += 1
                else:
                    break
            if first_bar_sem is None:
                continue  # no barrier prefix for this engine
            run = []
            while k < len(idxs):
                ins = insts[idxs[k]]
                si = getattr(ins, "sync_info", None)
                has_wait = si is not None and len(si.on_wait) > 0
                if (
                    getattr(ins, "opcode", "")
                    in ("DMACopy", "Matmult", "LoadActFuncSet")
                    and not has_wait
                ):
                    run.append(idxs[k])
                    k += 1
                else:
                    break
            if run:
                moved.append((first_bar_sem, run))
        if moved:
            pulled = {i for _, run in moved for i in run}
            inserts = {bidx: run for bidx, run in moved}
            new = []
            for idx, ins in enumerate(insts):
                if idx in pulled:
                    continue
                if idx in inserts:
                    new.extend(insts[i] for i in inserts[idx])
                new.append(ins)
            blk.instructions = new
    return nc


def _get_program(dt_in, raw=True):
    key = (str(dt_in),)
    if key not in _programs:
        _programs[key] = _build_raw(dt_in)
    return _programs[key]


def _fold_tables(Cm, XFc, XFs, D_val, D_row, D_col):
    """A[mi] = Cm[mi] * XF_mi @ Dblk_mi.T in float64 -> [128, 128, 64]."""
    Cm = np.asarray(Cm, np.float64)
    XFc = np.asarray(XFc, np.float64)
    XFs = np.asarray(XFs, np.float64)
    vals = np.asarray(D_val, np.float64)
    rows = np.asarray(D_row, np.int64)
    cols = np.asarray(D_col, np.int64)

    mi = rows // B
    l = rows - mi * B
    n = cols - mi * (2 * B)
    Dt = np.zeros((M, 2 * B, B))  # [mi, n, l] = Dblk_mi.T
    Dt[mi, n, l] = vals

    A = np.zeros((P, P, B))  # padded to 128 blocks; A[127] stays 0
    # B-1 = 63 is odd -> cos rows are the odd mi, sin rows the even mi
    A[0:M:2] = np.einsum("nk,mkl->mnl", XFs, Dt[0::2], optimize=True)
    A[1:M:2] = np.einsum("nk,mkl->mnl", XFc, Dt[1::2], optimize=True)
    A[:M] *= Cm[:, None, None]
    return A


def _np_dtype(dt_in):
    return mybir.dt.np(dt_in)


def _run(psiHat, A, trace=False, dt_in=DT_IN, raw=True):
    dt_np = _np_dtype(dt_in)
    # [b, m, n] -> [m, n, b], contiguous
    PT = np.ascontiguousarray(psiHat.transpose(1, 2, 0).astype(np.float32))

    in_maps = []
    for k in range(NCORES):
        mi0 = JPC * k
        nj = min(JPC, M - mi0)
        xt_k = np.zeros((P, JPC, NB), dt_np)
        xt_k[:, :nj, :] = PT[mi0 : mi0 + nj].transpose(1, 0, 2)
        a_k = np.zeros((P, JPC, B), dt_np)
        a_k[:, :nj, :] = A[mi0 : mi0 + nj].transpose(1, 0, 2)  # [n, nj, 64]
        in_maps.append(
            {"xt": xt_k.reshape(P, JPC * NB), "av": a_k.reshape(P, JPC * B)}
        )

    nc = _get_program(dt_in)
    res = run_bass_kernel_spmd(nc, in_maps, list(range(NCORES)), trace=trace)

    out = np.empty((NB, M, B), np.float32)
    for k in range(NCORES):
        mi0 = JPC * k
        nj = min(JPC, M - mi0)
        o = np.asarray(res.results[k]["out"]).reshape(2, B, PAIRS, NB)  # [h,l,p,b]
        ot = o.transpose(2, 0, 1, 3).reshape(JPC, B, NB)  # [j, l, b]
        out[:, mi0 : mi0 + nj, :] = ot[:nj].transpose(2, 0, 1)
    return out, res.exec_time_ns


def kernel(psiHat, Cm, XFc, XFs, D_val, D_row, D_col):
    psiHat = np.asarray(psiHat)
    A = _fold_tables(Cm, XFc, XFs, D_val, D_row, D_col)
    return _run(psiHat, A, trace=False)[0]


# revision 25
# speedup vs baseline: 1.2234x; 1.0528x over previous
"""Trainium2 Bass kernel for nn_FDLT (forward discrete Legendre transform).

Math: for each of the 127 m-blocks, the reference does
    out[:, mi, :] = (Cm[mi] * psiHat[:, mi, :]) @ XF_mi @ Dblk_mi.T
where XF_mi alternates XFc/XFs by mi parity and Dblk_mi is the mi-th
block of the block-diagonal sparse Wigner matrix D.  All tables are
runtime constants, so fold them on the host into A_mi = Cm[mi] * XF_mi
@ Dblk_mi.T (shape [128, 64]) and the device work collapses to 127
independent [512,128]@[128,64] matmuls.

Sharding: m-parallel across 8 cores (16 blocks/core, padded 128 with a
zero block), full batch per core.  The host feeds each core its input
slab pre-transposed to [n, j, b] so the contraction dim n lands on the
SBUF partition axis.  Block pairs write one shared [128, 512] PSUM bank
through PE quadrant placement: even block j -> out rows 0:64 with the
compact [128, 64] stationary at tile column 0, odd j -> rows 64:128 at
tile column 64 (tile_position).  Weights stay compact ([128, 16*64],
256 KB) instead of zero-padded pairs, halving weight DMA and keeping
the weight transfer off the input-stream critical path.  PSUM->SBUF
copies and output stores still run at the full 128 partitions.

Device I/O is fp16 (fp32 PSUM accumulation), measured ~3e-4 relative
error against the fp32 reference.
"""

from contextlib import ExitStack

import numpy as np

import concourse.bacc as bacc
import concourse.bass as bass  # noqa: F401
import concourse.mybir as mybir
from concourse.bass_utils import run_bass_kernel_spmd

P = 128      # SBUF partitions = n dim (2B)
B = 64       # l dim per block
M = 127      # number of m blocks
NB = 512     # full batch
NCORES = 8
JPC = 16     # m-blocks per core (8*16 = 128 = 127 real + 1 zero pad)
PAIRS = JPC // 2
# Input slab schedule (blocks per DMA).  Small head slabs get the first
# completion receipts to the PE sooner (receipts lag the data by
# ~2.5 us); small tail slabs shorten the post-last-byte chain.
SLABS = (1, 1, 2, 2, 2, 2, 2, 2, 1, 1)
WARMUP = 4   # garbage matmuls to ungate the PE clock before real work

# fp16 keeps a 10-bit mantissa (measured ~3e-4 relative error vs the
# fp32 reference with fp32-PSUM accumulation) while halving the DMA
# traffic that bounds this kernel.
DT_IN = mybir.dt.float16

_programs = {}


def _build_raw(dt_in):
    """Raw-bass pipeline with explicit semaphores, emitted blockless.

    Engine roles: Scalar loads the weights + input slab 0, then streams
    the output stores; Sync and GpSimd split the remaining input slabs
    between their two DMA rings; Tensor warms the PE clock gate with
    garbage matmuls, then runs the 16 real matmuls; Vector packs PSUM
    banks into SBUF staging.

    Semaphore soundness: a dma `then_inc(sem, 16)` arrives as 16
    independent +1s (one per SDMA engine), so a single cumulative sem
    across several DMAs can reach 16*k with one slow engine still
    mid-transfer on an early DMA.  Every DMA whose completion anything
    waits on therefore gets its own semaphore (per-slab s_in[i], s_av),
    and PSUM/output staging buffers are not reused (8 pairs = 8 PSUM
    banks + 8 staging tiles), killing all reuse waits.  s_mm/s_cp are
    single-producer compute sems (in-order increments), safe to wait on
    cumulatively.  The kernel ends with a full store-completion wait
    so no DMA is in flight when the NEFF epilogue runs.

    After compile, the unused const-AP memsets of the Bass preamble are
    stripped from the BIR, and each engine's leading run of wait-free
    input DMA issues is hoisted above its init-barrier entry so the
    transfers start while the Tensor sequencer is still waking up.  The
    barrier itself is kept (builds without it intermittently crash the
    device at a later process load).
    """
    assert dt_in != mybir.dt.float32r, "f32r path removed (compact lhsT only)"
    # 16-bit input -> store the output in fp16 too (host upcasts); the
    # extra ~2.4e-4 relative rounding halves the dominant output traffic.
    dt_out = (
        mybir.dt.float16
        if dt_in in (mybir.dt.float16, mybir.dt.bfloat16)
        else mybir.dt.float32
    )

    nc = bacc.Bacc(
        "TRN2", target_bir_lowering=False, debug=False, num_devices=NCORES
    )
    xt = nc.dram_tensor("xt", [P, JPC * NB], dt_in, kind="ExternalInput")
    av = nc.dram_tensor("av", [P, JPC * B], dt_in, kind="ExternalInput")
    out = nc.dram_tensor("out", [P, PAIRS * NB], dt_out, kind="ExternalOutput")

    assert sum(SLABS) == JPC
    NSLAB = len(SLABS)
    slab_of = []  # block j -> slab index
    slab_starts = []
    pos = 0
    for si, w in enumerate(SLABS):
        slab_starts.append(pos)
        slab_of.extend([si] * w)
        pos += w

    with ExitStack() as ctx:
        x_sb = ctx.enter_context(nc.sbuf_tensor("x_sb", [P, JPC * NB], dt_in))
        a_sb = ctx.enter_context(nc.sbuf_tensor("a_sb", [P, JPC * B], dt_in))
        o_sb = [
            ctx.enter_context(nc.sbuf_tensor(f"o_sb{i}", [P, NB], dt_out))
            for i in range(PAIRS)
        ]
        ps = [
            ctx.enter_context(
                nc.psum_tensor(f"ps{i}", [P, NB], mybir.dt.float32)
            )
            for i in range(PAIRS)
        ]
        s_in = [
            ctx.enter_context(nc.semaphore(f"s_in{i}")) for i in range(NSLAB)
        ]
        s_av = ctx.enter_context(nc.semaphore("s_av"))
        s_mm = ctx.enter_context(nc.semaphore("s_mm"))
        s_cpv = ctx.enter_context(nc.semaphore("s_cpv"))
        s_cps = ctx.enter_context(nc.semaphore("s_cps"))
        s_st = ctx.enter_context(nc.semaphore("s_st"))

        # --- Input issue plan: spread the ~0.65 us/DMA issue cost over
        # three DMA-capable engines (three rings) so every slab is issued
        # early and the SDMA engines can stream back-to-back.
        def _slab_dma(eng, s):
            lo = slab_starts[s] * NB
            hi = lo + SLABS[s] * NB
            eng.dma_start(out=x_sb[:, lo:hi], in_=xt[:, lo:hi]).then_inc(
                s_in[s], 16
            )

        nc.scalar.dma_start(out=a_sb[:], in_=av[:]).then_inc(s_av, 16)
        _slab_dma(nc.scalar, 0)
        for s in range(1, NSLAB, 2):
            _slab_dma(nc.sync, s)
        for s in range(2, NSLAB, 2):
            _slab_dma(nc.gpsimd, s)

        # --- Tensor: PE clock warm-up, then the 16 real matmuls.  Each
        # block is a [128, 64] compact stationary placed at PE tile
        # column 0 (even j -> PSUM rows 0:64) or 64 (odd j -> rows
        # 64:128); the pair shares one [128, 512] PSUM bank.
        for w in range(WARMUP):
            nc.tensor.matmul(
                ps[0][0:B, :],
                lhsT=a_sb[:, 0:B],
                rhs=x_sb[:, (JPC - 1) * NB : JPC * NB],
                start=True,
                stop=True,
                tile_position=(0, 0),
                skip_group_check=True,
            )
        nc.tensor.wait_ge(s_av, 16)
        for j in range(JPC):
            if j in slab_starts:
                nc.tensor.wait_ge(s_in[slab_of[j]], 16)
            p = j // 2
            half = ps[p][0:B, :] if j % 2 == 0 else ps[p][B:P, :]
            nc.tensor.matmul(
                half,
                lhsT=a_sb[:, j * B : (j + 1) * B],
                rhs=x_sb[:, j * NB : (j + 1) * NB],
                start=True,
                stop=True,
                tile_position=(0, 0 if j % 2 == 0 else B),
            ).then_inc(s_mm, 1)

        # --- PSUM -> SBUF staging, split across Vector (even pairs, sem
        # s_cpv) and Scalar's activation-copy (odd pairs, sem s_cps) so
        # the cast tail after the last receipt is not serialized on one
        # engine.  (GpSimd cannot read PSUM.)  The last pair is further
        # split into halves so its store can begin while the second half
        # converts.
        lp = PAIRS - 1
        h = NB // 2
        for p in range(PAIRS - 1):
            if p % 2 == 0:
                nc.vector.wait_ge(s_mm, 2 * p + 2)
                nc.vector.tensor_copy(o_sb[p][:], ps[p][:]).then_inc(
                    s_cpv, 1
                )
            else:
                nc.scalar.wait_ge(s_mm, 2 * p + 2)
                nc.scalar.copy(o_sb[p][:], ps[p][:]).then_inc(s_cps, 1)
        # Both halves on Scalar: a second engine on the same PSUM bank
        # (tried Vector for the second half) hangs the device — PSUM
        # banks do not support concurrent readers from two engines.
        nc.scalar.wait_ge(s_mm, 2 * lp + 2)
        nc.scalar.copy(o_sb[lp][:, 0:h], ps[lp][:, 0:h]).then_inc(s_cps, 1)
        nc.scalar.copy(o_sb[lp][:, h:NB], ps[lp][:, h:NB]).then_inc(s_cps, 1)

        # --- Output stores, split across Sync (even pairs) and GpSimd
        # (odd pairs); the last pair's two half-stores are issued from
        # both engines in parallel.
        for p in range(PAIRS - 1):
            if p % 2 == 0:
                nc.sync.wait_ge(s_cpv, p // 2 + 1)
                eng = nc.sync
            else:
                nc.gpsimd.wait_ge(s_cps, (p + 1) // 2)
                eng = nc.gpsimd
            eng.dma_start(
                out=out[:, p * NB : (p + 1) * NB], in_=o_sb[p][:]
            ).then_inc(s_st, 16)
        sc_n = (PAIRS - 1) // 2  # scalar full-pair casts before the halves
        nc.gpsimd.wait_ge(s_cps, sc_n + 1)
        nc.gpsimd.dma_start(
            out=out[:, lp * NB : lp * NB + h], in_=o_sb[lp][:, 0:h]
        ).then_inc(s_st, 16)
        nc.sync.wait_ge(s_cps, sc_n + 2)
        nc.sync.dma_start(
            out=out[:, lp * NB + h : (lp + 1) * NB], in_=o_sb[lp][:, h:NB]
        ).then_inc(s_st, 16)
        # Wait for every store to land before the kernel ends so no DMA
        # is in flight when the NEFF epilogue runs.  (Tried relying on
        # the NEFF epilogue's fini Drains instead: the exec dies with a
        # runtime INTERNAL error, so the explicit quiesce is required.)
        nc.scalar.wait_ge(s_st, 16 * (PAIRS + 1))

    nc.compile()

    # Strip only the unused const-AP memsets from the Bass preamble.  The
    # init all-engine barrier MUST stay: builds without it intermittently
    # leave the device unrecoverable at a subsequent fresh-process load
    # (~1-in-6 launches, observed twice), even with the store-quiesce
    # wait in place.
    for blk in nc.m.functions[0].blocks:
        blk.instructions = [
            i for i in blk.instructions if getattr(i, "opcode", "") != "Memset"
        ]

    # Hoist each engine's leading run of wait-free issues (DMA doorbells,
    # PE warm-up matmuls) above its init-barrier SEMAPHORE but below its
    # barrier DRAIN.  Drain blocks until the engine's outstanding DMAs
    # retire, so the issues must come after it; the barrier semaphore is
    # a pure sequencer sync, so work issued before it overlaps the slow
    # Tensor-sequencer wake-up that the barrier otherwise serializes
    # behind.  The semaphores the DMAs increment are initialized by the
    # NEFF loader, not by in-program clears, so pre-barrier issue is
    # sound.
    for blk in nc.m.functions[0].blocks:
        insts = blk.instructions
        per_eng = {}  # engine -> ordered instruction indices
        for idx, ins in enumerate(insts):
            eng = getattr(ins, "engine", None)
            if eng is not None:
                per_eng.setdefault(eng, []).append(idx)
        moved = []  # (insert_before_idx, [hoisted indices])
        for eng, idxs in per_eng.items():
            k = 0
            first_bar_sem = None
            # Skip the init-barrier prefix: Drain + barrier EventSemaphores.
            while k < len(idxs):
                ins = insts[idxs[k]]
                op = getattr(ins, "opcode", "")
                nm = getattr(ins, "name", "") or ""
                if op == "Drain":
                    k += 1
                elif op == "EventSemaphore" and nm.startswith("barrier_"):
                    if first_bar_sem is None:
                        first_bar_sem = idxs[k]
                    k += 1
                else:
                    break
            if first_bar_sem is None:
                continue  # no barrier prefix for this engine
            run = []
            while k < len(idxs):
                ins = insts[idxs[k]]
                si = getattr(ins, "sync_info", None)
                has_wait = si is not None and len(si.on_wait) > 0
                if (
                    getattr(ins, "opcode", "")
                    in ("DMACopy", "Matmult", "LoadActFuncSet")
                    and not has_wait
                ):
                    run.append(idxs[k])
                    k += 1
                else:
                    break
            if run:
                moved.append((first_bar_sem, run))
        if moved:
            pulled = {i for _, run in moved for i in run}
            inserts = {bidx: run for bidx, run in moved}
            new = []
            for idx, ins in enumerate(insts):
                if idx in pulled:
                    continue
                if idx in inserts:
                    new.extend(insts[i] for i in inserts[idx])
                new.append(ins)
            blk.instructions = new
    return nc


def _get_program(dt_in, raw=True):
    key = (str(dt_in),)
    if key not in _programs:
        _programs[key] = _build_raw(dt_in)
    return _programs[key]


def _fold_tables(Cm, XFc, XFs, D_val, D_row, D_col):
    """A[mi] = Cm[mi] * XF_mi @ Dblk_mi.T in float64 -> [128, 128, 64]."""
    Cm = np.asarray(Cm, np.float64)
    XFc = np.asarray(XFc, np.float64)
    XFs = np.asarray(XFs, np.float64)
    vals = np.asarray(D_val, np.float64)
    rows = np.asarray(D_row, np.int64)
    cols = np.asarray(D_col, np.int64)

    mi = rows // B
    l = rows - mi * B
    n = cols - mi * (2 * B)
    Dt = np.zeros((M, 2 * B, B))  # [mi, n, l] = Dblk_mi.T
    Dt[mi, n, l] = vals

    A = np.zeros((P, P, B))  # padded to 128 blocks; A[127] stays 0
    # B-1 = 63 is odd -> cos rows are the odd mi, sin rows the even mi
    A[0:M:2] = np.einsum("nk,mkl->mnl", XFs, Dt[0::2], optimize=True)
    A[1:M:2] = np.einsum("nk,mkl->mnl", XFc, Dt[1::2], optimize=True)
    A[:M] *= Cm[:, None, None]
    return A


def _np_dtype(dt_in):
    return mybir.dt.np(dt_in)


def _run(psiHat, A, trace=False, dt_in=DT_IN, raw=True):
    dt_np = _np_dtype(dt_in)
    # [b, m, n] -> [m, n, b], contiguous
    PT = np.ascontiguousarray(psiHat.transpose(1, 2, 0).astype(np.float32))

    in_maps = []
    for k in range(NCORES):
        mi0 = JPC * k
        nj = min(JPC, M - mi0)
        xt_k = np.zeros((P, JPC, NB), dt_np)
        xt_k[:, :nj, :] = PT[mi0 : mi0 + nj].transpose(1, 0, 2)
        a_k = np.zeros((P, JPC, B), dt_np)
        a_k[:, :nj, :] = A[mi0 : mi0 + nj].transpose(1, 0, 2)  # [n, nj, 64]
        in_maps.append(
            {"xt": xt_k.reshape(P, JPC * NB), "av": a_k.reshape(P, JPC * B)}
        )

    nc = _get_program(dt_in)
    res = run_bass_kernel_spmd(nc, in_maps, list(range(NCORES)), trace=trace)

    out = np.empty((NB, M, B), np.float32)
    for k in range(NCORES):
        mi0 = JPC * k
        nj = min(JPC, M - mi0)
        o = np.asarray(res.results[k]["out"]).reshape(2, B, PAIRS, NB)  # [h,l,p,b]
        ot = o.transpose(2, 0, 1, 3).reshape(JPC, B, NB)  # [j, l, b]
        out[:, mi0 : mi0 + nj, :] = ot[:nj].transpose(2, 0, 1)
    return out, res.exec_time_ns


def kernel(psiHat, Cm, XFc, XFs, D_val, D_row, D_col):
    psiHat = np.asarray(psiHat)
    A = _fold_tables(Cm, XFc, XFs, D_val, D_row, D_col)
    return _run(psiHat, A, trace=False)[0]
